# revision 22
# baseline (speedup 1.0000x reference)
"""Trainium2 Bass kernel for CenterWoParamMultiCosineSoftmaxLoss.

loss = mean_b sum_k softmax_k(2 - dst_bk) * dst_bk,
  dst_bk = 1 - <x_b/||x_b||, c_{l_b,k}/||c_{l_b,k}||>

Key identities used:
  softmax(2 - dst) = softmax(s)      (shift invariance; s = cosine score)
  per_sample       = 1 - sum_k p_k s_k
  s (normalized)   = raw_score * rnorm_b  (x-normalization folded in post-matmul)

Distribution: samples are grouped by label on the host (a sharding/layout
choice), padded into fixed 256-slot segments (one class per segment) so all
8 cores run one identical SPMD program; each core gets 12 segments (3072
slots) plus the raw center rows for those segments. Pad slots are zero rows
and contribute exactly 0 to the accumulated sum. All FLOPs (normalizations,
dot products, softmax, reductions) run on device.
"""

import sys

for _p in ("/opt/trn_rl_repo", "/root/.axon_site/_ro/trn_rl_repo"):
    if _p not in sys.path:
        sys.path.append(_p)

import numpy as np

import concourse.bass as bass
import concourse.mybir as mybir
from concourse.tile import TileContext
from concourse.masks import make_identity
from concourse.bass_utils import run_bass_kernel_spmd
from concourse.vector_clock import ScopedClock

B, D, C, K = 16384, 512, 90, 32
NCORES = 8
SEGW = 256          # slots per segment (one class per segment), 2 chunks of 128
P = 128
DCH = D // P        # 4 contraction chunks
f32 = mybir.dt.float32
bf16 = mybir.dt.bfloat16
AF = mybir.ActivationFunctionType
ALU = mybir.AluOpType

_tile_patched = False


def _install_tile_patch():
    """This walrus build allows only one sem wait on TPB_CTRL-lowered
    instructions (Drain / sync-NoOp). Tile's tail drain attaches one wait per
    live processor clock; split them into a chain of single-wait NoOps."""
    global _tile_patched
    if _tile_patched:
        return
    _tile_patched = True

    def _drain_and_barrier(self, tick_clock, wait_clock):
        nc = self.nc
        probe = nc.sync.nop(nofuse=True)
        wait_clock.add_sem_waits(
            probe.ins, ScopedClock({None: tick_clock.global_clock})
        )
        si = probe.ins.sync_info
        if si is not None and len(si.on_wait) > 1:
            waits = list(si.on_wait)
            si.on_wait.clear()
            si.on_wait.append(waits[0])
            for w in waits[1:]:
                n2 = nc.sync.nop(nofuse=True)
                if n2.ins.sync_info is None:
                    n2.ins.sync_info = mybir.SyncInfo(on_wait=[w], on_update=[])
                else:
                    n2.ins.sync_info.on_wait.append(w)
        nc.sync.drain()
        nc.all_engine_barrier()
        assert self.sems is not None
        popped = nc._tile_sem_poison_stack.pop()
        assert popped is self._sem_poison
        nc.clear_and_free_semaphores(list(self.sems.allocated().values()))
        nc.all_engine_barrier()

    TileContext._drain_and_barrier = _drain_and_barrier


def _split_excess_waits(nc, max_waits=1):
    """This walrus build accepts at most one sem wait per instruction for
    several opcodes. Hoist excess waits onto single-wait NoOps emitted just
    before the instruction on the same engine (engine streams are serial, so
    semantics are preserved)."""
    n = 0
    for fn in nc.m.functions:
        for blk in fn.blocks:
            newl = []
            for inst in blk.instructions:
                si = getattr(inst, "sync_info", None)
                if si is not None and si.on_wait is not None and len(si.on_wait) > max_waits:
                    waits = list(si.on_wait)
                    keep = waits[-max_waits:]
                    extra = waits[:-max_waits]
                    si.on_wait.clear()
                    for w in keep:
                        si.on_wait.append(w)
                    for w in extra:
                        n += 1
                        newl.append(
                            mybir.InstNoOp(
                                name=f"{inst.name}-w{n}",
                                engine=inst.engine,
                                sync_info=mybir.SyncInfo(on_wait=[w], on_update=[]),
                                bass_nofuse=True,
                            )
                        )
                newl.append(inst)
            blk.instructions[:] = newl
    import os
    if os.environ.get("BASS_DEBUG_WAITS"):
        print(f"[split_excess_waits] inserted {n} NoOps", file=sys.stderr)
    return nc


def build_bass(nseg: int, split_waits: bool = True):
    """One core's program: nseg segments of SEGW class-grouped sample slots."""
    _install_tile_patch()
    slots = nseg * SEGW
    nch = slots // P                  # 128-row chunks of x
    ck = nseg * K                     # center rows used
    ct = (ck + P - 1) // P            # center row tiles
    ckp = ct * P                      # padded center rows

    nc = bass.Bass()
    xg = nc.dram_tensor("xg", [slots, D], f32, kind="ExternalInput")
    cent = nc.dram_tensor("cent", [ckp, D], f32, kind="ExternalInput")
    out = nc.dram_tensor("partial", [1, 1], f32, kind="ExternalOutput")

    with TileContext(nc) as tc:
        with (
            tc.tile_pool(name="const", bufs=1) as const_pool,
            tc.tile_pool(name="persist", bufs=1) as persist,
            tc.tile_pool(name="cin", bufs=1) as cin_pool,
            tc.tile_pool(name="cnb", bufs=8) as cnb_pool,
            tc.tile_pool(name="xf", bufs=6) as xf_pool,
            tc.tile_pool(name="esb", bufs=4) as esb_pool,
            tc.tile_pool(name="junk", bufs=2) as junk_pool,
            tc.tile_pool(name="jk32", bufs=4) as jk32_pool,
            tc.tile_pool(name="tp_ps", bufs=3, space="PSUM") as tp_psum,
            tc.tile_pool(name="sc_ps", bufs=1, space="PSUM") as sc_psum,
            tc.tile_pool(name="fin_ps", bufs=1, space="PSUM") as fin_psum,
        ):
            id_f32 = const_pool.tile([P, P], f32)
            make_identity(nc, id_f32[:])
            id_bf16 = const_pool.tile([P, P], bf16)
            make_identity(nc, id_bf16[:])
            ones = const_pool.tile([P, 1], f32)
            nc.gpsimd.memset(ones[:], 1.0)

            # persistent tensors
            xT = persist.tile([P, DCH * slots], bf16)     # x^T, d-chunk c at cols [c*slots, +slots)
            cnT = persist.tile([P, DCH * ckp], bf16)      # cn^T, d-chunk c at cols [c*ckp, +ckp)
            mv = persist.tile([P, 2 * nch], f32)          # (mean, var) per chunk col
            rnorm = persist.tile([P, nch], f32)           # rsqrt(ss_x + eps)
            zsum = persist.tile([P, nch], f32)            # softmax denominators
            nums = persist.tile([P, nch], f32)            # sum_k e_k * s_raw_k
            c_ssr = persist.tile([P, ct], f32)            # 1/(ss_c + eps)
            c_rn = persist.tile([P, ct], f32)             # rsqrt(ss_c + eps)

            # ---- centers: load + row sum-of-squares ----
            cfs = []
            for t in range(ct):
                cf = cin_pool.tile([P, D], f32, tag=f"cin{t}")
                cfs.append(cf)
                nc.sync.dma_start(out=cf[:], in_=cent[t * P:(t + 1) * P, :])
                cjunk = junk_pool.tile([P, D], f32, tag="junk")
                # ss_c = sum_d c^2 via ACT Square+accum
                nc.scalar.activation(
                    out=cjunk[:], in_=cf[:], func=AF.Square,
                    accum_out=c_ssr[:, t:t + 1],
                )
            # batched center rsqrt: c_rn = exp(-0.5*ln(ss + eps))
            # (keeps ACT on the single natural_log_exp table: no table reloads)
            nc.vector.tensor_scalar_add(out=c_ssr[:], in0=c_ssr[:], scalar1=1e-12)
            c_ln = persist.tile([P, ct], f32)
            nc.scalar.activation(out=c_ln[:], in_=c_ssr[:], func=AF.Ln)
            nc.scalar.activation(out=c_rn[:], in_=c_ln[:], func=AF.Exp, scale=-0.5)
            # normalize + cast + transpose centers
            for t in range(ct):
                cb = cnb_pool.tile([P, D], bf16, tag="cnb")
                nc.scalar.activation(
                    out=cb[:], in_=cfs[t][:], func=AF.Copy, scale=c_rn[:, t:t + 1],
                )
                for c in range(DCH):
                    cps = tp_psum.tile([P, P], bf16, tag="tp")
                    nc.tensor.transpose(cps[:], cb[:, c * P:(c + 1) * P], id_bf16[:])
                    nc.vector.tensor_copy(
                        out=cnT[:, c * ckp + t * P: c * ckp + (t + 1) * P],
                        in_=cps[:],
                    )

            # ---- per-chunk: load x, norms, transpose, matmul, softmax ----
            # Scores are packed per group of GRP chunks into one PSUM bank
            # (disjoint 32-col slices) so PE never stalls on score-tile slots,
            # and the softmax for group g streams as soon as the group's
            # matmuls + rnorm are done.
            # cap the number of score PSUM banks at 4 (3 transpose + 4 score
            # + 1 final = 8 banks) for any nseg the packing produces
            GRP = max(6, (nch + 3) // 4)
            ngrp = (nch + GRP - 1) // GRP
            mv3 = mv[:].rearrange("p (i two) -> p i two", two=2)
            q = persist.tile([P, nch], f32)
            qln = persist.tile([P, nch], f32)
            scps = []
            for g in range(ngrp):
                scp_g = sc_psum.tile([P, GRP * K], f32, tag=f"scp{g}")
                scps.append(scp_g)
            for g in range(ngrp):
                chunks = range(g * GRP, min((g + 1) * GRP, nch))
                for i in chunks:
                    xf = xf_pool.tile([P, D], f32, tag="xf")
                    nc.sync.dma_start(out=xf[:], in_=xg[i * P:(i + 1) * P, :])

                    # mean/var over d in one DVE pass; ss = D*(var + mean^2)
                    bns = jk32_pool.tile([P, 6], f32, tag="bns")
                    nc.vector.bn_stats(out=bns[:], in_=xf[:])
                    nc.vector.bn_aggr(out=mv[:, 2 * i:2 * i + 2], in_=bns[:])

                    tps = tp_psum.tile([P, D], f32, tag="tp")
                    for c in range(DCH):
                        nc.tensor.transpose(
                            tps[:, c * P:(c + 1) * P], xf[:, c * P:(c + 1) * P],
                            id_f32[:],
                        )
                    # pack all 4 d-blocks of this chunk into xT via one copy+cast
                    xt_dst = xT[:].rearrange("p (c n) -> p c n", c=DCH)[
                        :, :, i * P:(i + 1) * P
                    ]
                    tps_src = tps[:].rearrange("p (c n) -> p c n", c=DCH)
                    nc.scalar.activation(out=xt_dst, in_=tps_src, func=AF.Copy)

                    # scores for this chunk's class j = i // (SEGW // P)
                    j = i // (SEGW // P)
                    sc = scps[g][:, (i - g * GRP) * K:(i - g * GRP + 1) * K]
                    for c in range(DCH):
                        nc.tensor.matmul(
                            sc,
                            xT[:, c * slots + i * P: c * slots + (i + 1) * P],
                            cnT[:, c * ckp + j * K: c * ckp + (j + 1) * K],
                            start=(c == 0),
                            stop=(c == DCH - 1),
                        )

                # group rnorm = 1/sqrt(D*(var + mean^2) + eps) via ln/exp
                c0, c1 = g * GRP, min((g + 1) * GRP, nch)
                qg = q[:, c0:c1]
                qg3 = q[:].rearrange("p (i one) -> p i one", one=1)[:, c0:c1]
                nc.vector.tensor_mul(
                    out=qg3, in0=mv3[:, c0:c1, 0:1], in1=mv3[:, c0:c1, 0:1]
                )
                nc.vector.tensor_add(out=qg3, in0=qg3, in1=mv3[:, c0:c1, 1:2])
                nc.vector.tensor_scalar(
                    out=qg, in0=qg, scalar1=float(D), scalar2=1e-12,
                    op0=ALU.mult, op1=ALU.add,
                )
                nc.scalar.activation(out=qln[:, c0:c1], in_=qg, func=AF.Ln)
                nc.scalar.activation(
                    out=rnorm[:, c0:c1], in_=qln[:, c0:c1], func=AF.Exp, scale=-0.5
                )

                # softmax over K, batched across the group's chunks:
                # ssc = s_raw * rnorm (per-chunk scale), e = exp(ssc) in one
                # ACT op, Z and num = sum_k e*ssc via segmented DVE reduces.
                # Then t = num/Z directly (the rnorm factor is inside ssc).
                gw = len(chunks)
                ssc = esb_pool.tile([P, GRP * K], f32, tag="ssc")
                for i in chunks:
                    ii = i - g * GRP
                    nc.vector.tensor_scalar_mul(
                        out=ssc[:, ii * K:(ii + 1) * K],
                        in0=scps[g][:, ii * K:(ii + 1) * K],
                        scalar1=rnorm[:, i:i + 1],
                    )
                e = esb_pool.tile([P, GRP * K], f32, tag="esb")
                nc.scalar.activation(
                    out=e[:, :gw * K], in_=ssc[:, :gw * K], func=AF.Exp,
                )
                e3 = e[:].rearrange("p (i k) -> p i k", k=K)
                nc.vector.tensor_reduce(
                    out=zsum[:, c0:c1], in_=e3[:, :gw],
                    axis=mybir.AxisListType.X, op=ALU.add,
                )
                jk = jk32_pool.tile([P, GRP * K], f32, tag="jk32")
                nc.vector.tensor_mul(
                    out=jk[:, :gw * K], in0=e[:, :gw * K], in1=ssc[:, :gw * K]
                )
                jk3 = jk[:].rearrange("p (i k) -> p i k", k=K)
                nc.vector.tensor_reduce(
                    out=nums[:, c0:c1], in_=jk3[:, :gw],
                    axis=mybir.AxisListType.X, op=ALU.add,
                )

            # ---- tail: t = num / Z, partial = sum over all slots ----
            nc.vector.reciprocal(out=zsum[:], in_=zsum[:])
            nc.vector.tensor_mul(out=nums[:], in0=nums[:], in1=zsum[:])
            red = persist.tile([P, 1], f32)
            nc.vector.tensor_reduce(
                out=red[:], in_=nums[:], axis=mybir.AxisListType.X, op=ALU.add,
            )
            fin = fin_psum.tile([1, 1], f32)
            nc.tensor.matmul(fin[:], red[:], ones[:], start=True, stop=True)
            osb = const_pool.tile([1, 1], f32)
            nc.scalar.copy(out=osb[:], in_=fin[:])
            nc.sync.dma_start(out=out[:], in_=osb[:])

    if split_waits:
        _split_excess_waits(nc)
    return nc


def _pack_segments(labels: np.ndarray):
    """Group sample indices by label into segments of <= SEGW, one class per
    segment; pad total segment count to a multiple of NCORES."""
    order = np.argsort(labels, kind="stable")
    sorted_lab = labels[order]
    # boundaries of equal-label runs
    cut = np.flatnonzero(np.diff(sorted_lab)) + 1
    starts = np.concatenate(([0], cut))
    ends = np.concatenate((cut, [len(labels)]))
    segs = []  # (class, sample_index_array)
    for s, e in zip(starts, ends):
        cls = int(sorted_lab[s])
        for o in range(s, e, SEGW):
            segs.append((cls, order[o:min(o + SEGW, e)]))
    while len(segs) % NCORES != 0:
        segs.append((0, np.empty(0, dtype=np.int64)))
    return segs


def kernel(x: np.ndarray, labels: np.ndarray, centers: np.ndarray) -> np.ndarray:
    x = np.ascontiguousarray(x, dtype=np.float32)
    labels = np.asarray(labels)
    centers = np.ascontiguousarray(centers, dtype=np.float32)
    nb, d = x.shape
    ncls, k, _ = centers.shape
    assert (nb, d, k) == (B, D, K)

    segs = _pack_segments(labels)
    nseg_total = len(segs)
    nseg = nseg_total // NCORES
    slots = nseg * SEGW
    ck = nseg * K
    ckp = ((ck + P - 1) // P) * P

    in_maps = []
    for core in range(NCORES):
        xg = np.zeros((slots, d), dtype=np.float32)
        cent = np.zeros((ckp, d), dtype=np.float32)
        for jj in range(nseg):
            cls, idx = segs[core * nseg + jj]
            if len(idx):
                xg[jj * SEGW: jj * SEGW + len(idx)] = x[idx]
            cent[jj * K:(jj + 1) * K] = centers[cls]
        in_maps.append({"xg": xg, "cent": cent})

    nc = build_bass(nseg)
    res = run_bass_kernel_spmd(nc, in_maps, core_ids=list(range(NCORES)))
    total = sum(float(r["partial"][0, 0]) for r in res.results)
    return np.float32(1.0 - total / nb)


# revision 23
# speedup vs baseline: 1.0743x; 1.0743x over previous
"""Trainium2 Bass kernel for CenterWoParamMultiCosineSoftmaxLoss.

loss = mean_b sum_k softmax_k(2 - dst_bk) * dst_bk,
  dst_bk = 1 - <x_b/||x_b||, c_{l_b,k}/||c_{l_b,k}||>

Key identities used:
  softmax(2 - dst) = softmax(s)      (shift invariance; s = cosine score)
  per_sample       = 1 - sum_k p_k s_k
  s (normalized)   = raw_score * rnorm_b  (x-normalization folded in post-matmul)

Distribution: samples are grouped by label on the host (a sharding/layout
choice), padded into fixed 256-slot segments (one class per segment) so all
8 cores run one identical SPMD program; each core gets 12 segments (3072
slots) plus the raw center rows for those segments. Pad slots are zero rows
and contribute exactly 0 to the accumulated sum. All FLOPs (normalizations,
dot products, softmax, reductions) run on device.
"""

import sys

for _p in ("/opt/trn_rl_repo", "/root/.axon_site/_ro/trn_rl_repo"):
    if _p not in sys.path:
        sys.path.append(_p)

import numpy as np

import concourse.bass as bass
import concourse.mybir as mybir
from concourse.tile import TileContext
from concourse.masks import make_identity
from concourse.bass_utils import run_bass_kernel_spmd
from concourse.vector_clock import ScopedClock

B, D, C, K = 16384, 512, 90, 32
NCORES = 8
SEGW = 256          # slots per segment (one class per segment), 2 chunks of 128
P = 128
DCH = D // P        # 4 contraction chunks
f32 = mybir.dt.float32
bf16 = mybir.dt.bfloat16
AF = mybir.ActivationFunctionType
ALU = mybir.AluOpType

_tile_patched = False


def _install_tile_patch():
    """This walrus build allows only one sem wait on TPB_CTRL-lowered
    instructions (Drain / sync-NoOp). Tile's tail drain attaches one wait per
    live processor clock; split them into a chain of single-wait NoOps."""
    global _tile_patched
    if _tile_patched:
        return
    _tile_patched = True

    def _drain_and_barrier(self, tick_clock, wait_clock):
        nc = self.nc
        probe = nc.sync.nop(nofuse=True)
        wait_clock.add_sem_waits(
            probe.ins, ScopedClock({None: tick_clock.global_clock})
        )
        si = probe.ins.sync_info
        if si is not None and len(si.on_wait) > 1:
            waits = list(si.on_wait)
            si.on_wait.clear()
            si.on_wait.append(waits[0])
            for w in waits[1:]:
                n2 = nc.sync.nop(nofuse=True)
                if n2.ins.sync_info is None:
                    n2.ins.sync_info = mybir.SyncInfo(on_wait=[w], on_update=[])
                else:
                    n2.ins.sync_info.on_wait.append(w)
        nc.sync.drain()
        nc.all_engine_barrier()
        assert self.sems is not None
        popped = nc._tile_sem_poison_stack.pop()
        assert popped is self._sem_poison
        nc.clear_and_free_semaphores(list(self.sems.allocated().values()))
        nc.all_engine_barrier()

    TileContext._drain_and_barrier = _drain_and_barrier


def _split_excess_waits(nc, max_waits=1):
    """This walrus build accepts at most one sem wait per instruction for
    several opcodes. Hoist excess waits onto single-wait NoOps emitted just
    before the instruction on the same engine (engine streams are serial, so
    semantics are preserved)."""
    n = 0
    for fn in nc.m.functions:
        for blk in fn.blocks:
            newl = []
            for inst in blk.instructions:
                si = getattr(inst, "sync_info", None)
                if si is not None and si.on_wait is not None and len(si.on_wait) > max_waits:
                    waits = list(si.on_wait)
                    keep = waits[-max_waits:]
                    extra = waits[:-max_waits]
                    si.on_wait.clear()
                    for w in keep:
                        si.on_wait.append(w)
                    for w in extra:
                        n += 1
                        newl.append(
                            mybir.InstNoOp(
                                name=f"{inst.name}-w{n}",
                                engine=inst.engine,
                                sync_info=mybir.SyncInfo(on_wait=[w], on_update=[]),
                                bass_nofuse=True,
                            )
                        )
                newl.append(inst)
            blk.instructions[:] = newl
    import os
    if os.environ.get("BASS_DEBUG_WAITS"):
        print(f"[split_excess_waits] inserted {n} NoOps", file=sys.stderr)
    return nc


def build_bass(nseg: int, split_waits: bool = True):
    """One core's program: nseg segments of SEGW class-grouped sample slots."""
    _install_tile_patch()
    slots = nseg * SEGW
    nch = slots // P                  # 128-row chunks of x
    ck = nseg * K                     # center rows used
    ct = (ck + P - 1) // P            # center row tiles
    ckp = ct * P                      # padded center rows

    nc = bass.Bass()
    xg = nc.dram_tensor("xg", [slots, D], f32, kind="ExternalInput")
    cent = nc.dram_tensor("cent", [ckp, D], f32, kind="ExternalInput")
    out = nc.dram_tensor("partial", [1, 1], f32, kind="ExternalOutput")

    with TileContext(nc) as tc:
        with (
            tc.tile_pool(name="const", bufs=1) as const_pool,
            tc.tile_pool(name="persist", bufs=1) as persist,
            tc.tile_pool(name="cin", bufs=1) as cin_pool,
            tc.tile_pool(name="cnb", bufs=8) as cnb_pool,
            tc.tile_pool(name="xf", bufs=8) as xf_pool,
            tc.tile_pool(name="esb", bufs=4) as esb_pool,
            tc.tile_pool(name="junk", bufs=2) as junk_pool,
            tc.tile_pool(name="jk32", bufs=4) as jk32_pool,
            tc.tile_pool(name="tp_ps", bufs=3, space="PSUM") as tp_psum,
            tc.tile_pool(name="sc_ps", bufs=1, space="PSUM") as sc_psum,
            tc.tile_pool(name="fin_ps", bufs=1, space="PSUM") as fin_psum,
        ):
            id_f32 = const_pool.tile([P, P], f32)
            make_identity(nc, id_f32[:])
            id_bf16 = const_pool.tile([P, P], bf16)
            make_identity(nc, id_bf16[:])
            ones = const_pool.tile([P, 1], f32)
            nc.gpsimd.memset(ones[:], 1.0)

            # persistent tensors
            xT = persist.tile([P, DCH * slots], bf16)     # x^T, d-chunk c at cols [c*slots, +slots)
            cnT = persist.tile([P, DCH * ckp], bf16)      # cn^T, d-chunk c at cols [c*ckp, +ckp)
            mv = persist.tile([P, 2 * nch], f32)          # (mean, var) per chunk col
            rnorm = persist.tile([P, nch], f32)           # rsqrt(ss_x + eps)
            zsum = persist.tile([P, nch], f32)            # softmax denominators
            nums = persist.tile([P, nch], f32)            # sum_k e_k * s_raw_k
            c_ssr = persist.tile([P, ct], f32)            # 1/(ss_c + eps)
            c_rn = persist.tile([P, ct], f32)             # rsqrt(ss_c + eps)

            # ---- centers: load + row sum-of-squares ----
            cfs = []
            for t in range(ct):
                cf = cin_pool.tile([P, D], f32, tag=f"cin{t}")
                cfs.append(cf)
                nc.sync.dma_start(out=cf[:], in_=cent[t * P:(t + 1) * P, :])
                cjunk = junk_pool.tile([P, D], f32, tag="junk")
                # ss_c = sum_d c^2 via ACT Square+accum
                nc.scalar.activation(
                    out=cjunk[:], in_=cf[:], func=AF.Square,
                    accum_out=c_ssr[:, t:t + 1],
                )
            # batched center rsqrt: c_rn = exp(-0.5*ln(ss + eps))
            # (keeps ACT on the single natural_log_exp table: no table reloads)
            nc.vector.tensor_scalar_add(out=c_ssr[:], in0=c_ssr[:], scalar1=1e-12)
            c_ln = persist.tile([P, ct], f32)
            nc.scalar.activation(out=c_ln[:], in_=c_ssr[:], func=AF.Ln)
            nc.scalar.activation(out=c_rn[:], in_=c_ln[:], func=AF.Exp, scale=-0.5)
            # normalize + cast + transpose centers
            for t in range(ct):
                cb = cnb_pool.tile([P, D], bf16, tag="cnb")
                nc.scalar.activation(
                    out=cb[:], in_=cfs[t][:], func=AF.Copy, scale=c_rn[:, t:t + 1],
                )
                for c in range(DCH):
                    cps = tp_psum.tile([P, P], bf16, tag="tp")
                    nc.tensor.transpose(cps[:], cb[:, c * P:(c + 1) * P], id_bf16[:])
                    nc.vector.tensor_copy(
                        out=cnT[:, c * ckp + t * P: c * ckp + (t + 1) * P],
                        in_=cps[:],
                    )

            # ---- per-chunk: load x, norms, transpose, matmul, softmax ----
            # Scores are packed per group of GRP chunks into one PSUM bank
            # (disjoint 32-col slices) so PE never stalls on score-tile slots,
            # and the softmax for group g streams as soon as the group's
            # matmuls + rnorm are done.
            # cap the number of score PSUM banks at 4 (3 transpose + 4 score
            # + 1 final = 8 banks) for any nseg the packing produces
            GRP = max(6, (nch + 3) // 4)
            ngrp = (nch + GRP - 1) // GRP
            mv3 = mv[:].rearrange("p (i two) -> p i two", two=2)
            q = persist.tile([P, nch], f32)
            qln = persist.tile([P, nch], f32)
            scps = []
            for g in range(ngrp):
                scp_g = sc_psum.tile([P, GRP * K], f32, tag=f"scp{g}")
                scps.append(scp_g)
            for g in range(ngrp):
                chunks = range(g * GRP, min((g + 1) * GRP, nch))
                for i in chunks:
                    xf = xf_pool.tile([P, D], f32, tag="xf")
                    nc.sync.dma_start(out=xf[:], in_=xg[i * P:(i + 1) * P, :])

                    # mean/var over d in one DVE pass; ss = D*(var + mean^2)
                    bns = jk32_pool.tile([P, 6], f32, tag="bns")
                    nc.vector.bn_stats(out=bns[:], in_=xf[:])
                    nc.vector.bn_aggr(out=mv[:, 2 * i:2 * i + 2], in_=bns[:])

                    tps = tp_psum.tile([P, D], f32, tag="tp")
                    for c in range(DCH):
                        nc.tensor.transpose(
                            tps[:, c * P:(c + 1) * P], xf[:, c * P:(c + 1) * P],
                            id_f32[:],
                        )
                    # pack all 4 d-blocks of this chunk into xT via one copy+cast
                    xt_dst = xT[:].rearrange("p (c n) -> p c n", c=DCH)[
                        :, :, i * P:(i + 1) * P
                    ]
                    tps_src = tps[:].rearrange("p (c n) -> p c n", c=DCH)
                    nc.scalar.activation(out=xt_dst, in_=tps_src, func=AF.Copy)

                    # scores for this chunk's class j = i // (SEGW // P)
                    j = i // (SEGW // P)
                    sc = scps[g][:, (i - g * GRP) * K:(i - g * GRP + 1) * K]
                    for c in range(DCH):
                        nc.tensor.matmul(
                            sc,
                            xT[:, c * slots + i * P: c * slots + (i + 1) * P],
                            cnT[:, c * ckp + j * K: c * ckp + (j + 1) * K],
                            start=(c == 0),
                            stop=(c == DCH - 1),
                        )

                # group rnorm = 1/sqrt(D*(var + mean^2) + eps) via ln/exp
                c0, c1 = g * GRP, min((g + 1) * GRP, nch)
                qg = q[:, c0:c1]
                qg3 = q[:].rearrange("p (i one) -> p i one", one=1)[:, c0:c1]
                nc.vector.tensor_mul(
                    out=qg3, in0=mv3[:, c0:c1, 0:1], in1=mv3[:, c0:c1, 0:1]
                )
                nc.vector.tensor_add(out=qg3, in0=qg3, in1=mv3[:, c0:c1, 1:2])
                nc.vector.tensor_scalar(
                    out=qg, in0=qg, scalar1=float(D), scalar2=1e-12,
                    op0=ALU.mult, op1=ALU.add,
                )
                nc.scalar.activation(out=qln[:, c0:c1], in_=qg, func=AF.Ln)
                nc.scalar.activation(
                    out=rnorm[:, c0:c1], in_=qln[:, c0:c1], func=AF.Exp, scale=-0.5
                )

                # softmax over K, batched across the group's chunks:
                # ssc = s_raw * rnorm (per-chunk scale), e = exp(ssc) in one
                # ACT op, Z and num = sum_k e*ssc via segmented DVE reduces.
                # Then t = num/Z directly (the rnorm factor is inside ssc).
                gw = len(chunks)
                ssc = esb_pool.tile([P, GRP * K], f32, tag="ssc")
                for i in chunks:
                    ii = i - g * GRP
                    # ACT (idle at the tail) applies the per-sample scale;
                    # DVE keeps only the reductions.
                    nc.scalar.activation(
                        out=ssc[:, ii * K:(ii + 1) * K],
                        in_=scps[g][:, ii * K:(ii + 1) * K],
                        func=AF.Copy,
                        scale=rnorm[:, i:i + 1],
                    )
                e = esb_pool.tile([P, GRP * K], f32, tag="esb")
                nc.scalar.activation(
                    out=e[:, :gw * K], in_=ssc[:, :gw * K], func=AF.Exp,
                )
                e3 = e[:].rearrange("p (i k) -> p i k", k=K)
                nc.vector.tensor_reduce(
                    out=zsum[:, c0:c1], in_=e3[:, :gw],
                    axis=mybir.AxisListType.X, op=ALU.add,
                )
                jk = jk32_pool.tile([P, GRP * K], f32, tag="jk32")
                nc.vector.tensor_mul(
                    out=jk[:, :gw * K], in0=e[:, :gw * K], in1=ssc[:, :gw * K]
                )
                jk3 = jk[:].rearrange("p (i k) -> p i k", k=K)
                nc.vector.tensor_reduce(
                    out=nums[:, c0:c1], in_=jk3[:, :gw],
                    axis=mybir.AxisListType.X, op=ALU.add,
                )

            # ---- tail: t = num / Z, partial = sum over all slots ----
            nc.vector.reciprocal(out=zsum[:], in_=zsum[:])
            nc.vector.tensor_mul(out=nums[:], in0=nums[:], in1=zsum[:])
            red = persist.tile([P, 1], f32)
            nc.vector.tensor_reduce(
                out=red[:], in_=nums[:], axis=mybir.AxisListType.X, op=ALU.add,
            )
            fin = fin_psum.tile([1, 1], f32)
            nc.tensor.matmul(fin[:], red[:], ones[:], start=True, stop=True)
            osb = const_pool.tile([1, 1], f32)
            nc.scalar.copy(out=osb[:], in_=fin[:])
            nc.sync.dma_start(out=out[:], in_=osb[:])

    if split_waits:
        _split_excess_waits(nc)
    return nc


def _pack_segments(labels: np.ndarray):
    """Group sample indices by label into segments of <= SEGW, one class per
    segment; pad total segment count to a multiple of NCORES."""
    order = np.argsort(labels, kind="stable")
    sorted_lab = labels[order]
    # boundaries of equal-label runs
    cut = np.flatnonzero(np.diff(sorted_lab)) + 1
    starts = np.concatenate(([0], cut))
    ends = np.concatenate((cut, [len(labels)]))
    segs = []  # (class, sample_index_array)
    for s, e in zip(starts, ends):
        cls = int(sorted_lab[s])
        for o in range(s, e, SEGW):
            segs.append((cls, order[o:min(o + SEGW, e)]))
    while len(segs) % NCORES != 0:
        segs.append((0, np.empty(0, dtype=np.int64)))
    return segs


def kernel(x: np.ndarray, labels: np.ndarray, centers: np.ndarray) -> np.ndarray:
    x = np.ascontiguousarray(x, dtype=np.float32)
    labels = np.asarray(labels)
    centers = np.ascontiguousarray(centers, dtype=np.float32)
    nb, d = x.shape
    ncls, k, _ = centers.shape
    assert (nb, d, k) == (B, D, K)

    segs = _pack_segments(labels)
    nseg_total = len(segs)
    nseg = nseg_total // NCORES
    slots = nseg * SEGW
    ck = nseg * K
    ckp = ((ck + P - 1) // P) * P

    in_maps = []
    for core in range(NCORES):
        xg = np.zeros((slots, d), dtype=np.float32)
        cent = np.zeros((ckp, d), dtype=np.float32)
        for jj in range(nseg):
            cls, idx = segs[core * nseg + jj]
            if len(idx):
                xg[jj * SEGW: jj * SEGW + len(idx)] = x[idx]
            cent[jj * K:(jj + 1) * K] = centers[cls]
        in_maps.append({"xg": xg, "cent": cent})

    nc = build_bass(nseg)
    res = run_bass_kernel_spmd(nc, in_maps, core_ids=list(range(NCORES)))
    total = sum(float(r["partial"][0, 0]) for r in res.results)
    return np.float32(1.0 - total / nb)
